# revision 1
# baseline (speedup 1.0000x reference)
"""Trainium2 Bass kernel for nn_CrossAttention (B=4, C=512, H=W=64, CQK=64).

Math (per batch b):
    Q = Wq @ rgb + bq                      [CQK, HW]
    K = Wk @ chm + bk                      [CQK, XY]
    V = Wv @ chm + bv                      [C, XY]
    S[hw, xy] = sum_o Q[o, hw] K[o, xy]    (xy = x*64 + y)
    P = softmax over y only (last 64-group of xy)
    att[c, hw] = sum_xy P[hw, xy] V[c, xy]
    out = rgb + gamma * att

Sharding: 8 cores = 4 batches x 2 halves of the hw (query) axis. Weights
replicated. Each core computes the full K/V for its batch and its 2048-row
slice of queries.

Device dataflow per core (all layouts channel/feature-major):
  - Qt[o, hw] (f32r), Kf[o, xy] (f32r) via 1x1-conv GEMMs; V^T[xy, c] (bf16).
  - S tiles [128 hw, xy] on PE (f32r), exp on ACT -> P (bf16, unnormalized),
    Z via DVE pairwise-tree sum over y, reciprocal, broadcast-multiply.
  - P^T via DMA xbar transpose (bf16), attend GEMM on PE (bf16),
    final add with rgb on DVE.
  - gamma and bv are folded on the host (bv contributes 64*gamma*bv[c] since
    softmax rows sum to 1 per (hw, x) and there are 64 x's).
DMA engine split: bulk loads on SWDGE (gpsimd), stores on the ACT HWDGE ring
(scalar), xbar transposes on the SP HWDGE ring (sync) to avoid single-FIFO
serialization.
"""

import numpy as np

import concourse.bass as bass
import concourse.mybir as mybir
import concourse.tile as tile
from concourse import bacc
from concourse.bass_utils import run_bass_kernel_spmd

P = 128
B, C, H, W = 4, 512, 64, 64
HW = H * W                # 4096
CQK = C // 8              # 64
N_CORES = 8
HWC = HW // 2             # hw rows per core (2048)

F32 = mybir.dt.float32
F32R = mybir.dt.float32r
BF16 = mybir.dt.bfloat16
ADD = mybir.AluOpType.add
MULT = mybir.AluOpType.mult
IDENT = mybir.ActivationFunctionType.Identity
EXP = mybir.ActivationFunctionType.Exp


def build_program(hwc=HWC, xy=HW, c=C, cqk=CQK, n_cores=N_CORES, repeat=1,
                  load_eng="gpsimd", store_eng="sync", ptb_bufs=2):
    """Build the per-core Bass program. Returns a compiled Bacc module."""
    ck = c // P               # channel chunks (4)
    nb = hwc // 512           # hw blocks (4)
    xt = xy // P              # xy tiles (32)
    xb = xy // 512            # xy 512-blocks (8)
    y = 64                    # softmax group size
    x_per_tile = xy // y      # x values (64 full size)

    nc = bacc.Bacc("TRN2", target_bir_lowering=False, debug=False,
                   num_devices=n_cores)
    ld = {"sync": nc.sync, "scalar": nc.scalar, "gpsimd": nc.gpsimd}[load_eng]
    st = {"sync": nc.sync, "scalar": nc.scalar, "gpsimd": nc.gpsimd}[store_eng]

    rgb = nc.dram_tensor("rgb", [c, hwc], F32, kind="ExternalInput")
    chm = nc.dram_tensor("chm", [c, xy], F32, kind="ExternalInput")
    wqT = nc.dram_tensor("wqT", [c, 2 * cqk], F32, kind="ExternalInput")
    wkT = nc.dram_tensor("wkT", [c, 2 * cqk], F32, kind="ExternalInput")
    wvT = nc.dram_tensor("wvT", [c, c], F32, kind="ExternalInput")
    bq = nc.dram_tensor("bq", [2 * cqk, 1], F32, kind="ExternalInput")
    bk = nc.dram_tensor("bk", [2 * cqk, 1], F32, kind="ExternalInput")
    out = nc.dram_tensor("out", [c, hwc], F32, kind="ExternalOutput")

    rgb_t = rgb.ap().rearrange("(k p) n -> p k n", p=P)
    chm_t = chm.ap().rearrange("(k p) n -> p k n", p=P)
    wq_t = wqT.ap().rearrange("(k p) m -> p k m", p=P)
    wk_t = wkT.ap().rearrange("(k p) m -> p k m", p=P)
    wv_t = wvT.ap().rearrange("(k p) m -> p k m", p=P)
    out_t = out.ap().rearrange("(k p) n -> p k n", p=P)

    with tile.TileContext(nc) as tc:
        with tc.tile_pool(name="persist", bufs=1) as pers:
            # --- weights / biases ---
            wq_r = pers.tile([P, ck, 2 * cqk], F32R)
            wk_r = pers.tile([P, ck, 2 * cqk], F32R)
            wv_b = pers.tile([P, ck, c], BF16)
            with tc.tile_pool(name="wload", bufs=1) as wload:
                wq_f = wload.tile([P, ck, 2 * cqk], F32)
                ld.dma_start(wq_f[:], wq_t)
                nc.vector.tensor_copy(wq_r[:], wq_f[:])
            bq_sb = pers.tile([2 * cqk, 1], F32)
            ld.dma_start(bq_sb[:], bq.ap())
            bk_sb = pers.tile([2 * cqk, 1], F32)
            ld.dma_start(bk_sb[:], bk.ap())

            qt_sb = pers.tile([2 * cqk, hwc], F32R)
            kf_sb = pers.tile([2 * cqk, xy], F32R)
            chmT_bf = pers.tile([P, xt, ck, P], BF16)

            for _rep in range(repeat):
                # deferred weight loads (not needed until Kf / att2)
                with tc.tile_pool(name="wload2", bufs=1) as wload2:
                    wk_f = wload2.tile([P, ck, 2 * cqk], F32)
                    ld.dma_start(wk_f[:], wk_t)
                    nc.vector.tensor_copy(wk_r[:], wk_f[:])
                    wv_f = wload2.tile([P, ck, c], F32)
                    ld.dma_start(wv_f[:], wv_t)
                    nc.vector.tensor_copy(wv_b[:], wv_f[:])

                # --- phase 1: Qt GEMM (rgb streamed) then Kf GEMM (chm
                # streamed); chm_bf shares the ptb tag: identical
                # 32KB/partition footprint, so phase 2's P^T buffers reuse its
                # slot once the chmT transposes are done.
                with tc.tile_pool(name="ptpool", bufs=ptb_bufs) as ptpool:
                    chm_bf = ptpool.tile([P, ck, xy], BF16, tag="ptb",
                                         name="chmbf")
                    half = xy // 2
                    with tc.tile_pool(name="qstream", bufs=2) as qstream, \
                         tc.tile_pool(name="psQ", bufs=1, space="PSUM") as psQ:
                        q_ps = [psQ.tile([2 * cqk, 512], F32, name=f"qps{i}")
                                for i in range(nb)]
                        for k in range(ck):
                            rf = qstream.tile([P, hwc], F32, tag="rf")
                            ld.dma_start(rf[:], rgb_t[:, k])
                            rr = qstream.tile([P, hwc], F32R, tag="rr")
                            nc.vector.tensor_copy(rr[:], rf[:])
                            for j in range(nb):
                                nc.tensor.matmul(
                                    q_ps[j][:], wq_r[:, k],
                                    rr[:, 512 * j:512 * (j + 1)],
                                    start=(k == 0), stop=(k == ck - 1))
                        for i in range(nb):
                            nc.scalar.activation(qt_sb[:, 512 * i:512 * (i + 1)],
                                                 q_ps[i][:], IDENT, bias=bq_sb[:])

                    with tc.tile_pool(name="stream", bufs=2) as stream, \
                         tc.tile_pool(name="psK", bufs=1, space="PSUM") as psK:
                        k_ps = [psK.tile([2 * cqk, 512], F32, name=f"kps{i}")
                                for i in range(xb)]
                        for k in range(ck):
                            for h in range(2):
                                cf = stream.tile([P, half], F32, tag="cf")
                                ld.dma_start(
                                    cf[:], chm_t[:, k, h * half:(h + 1) * half])
                                nc.scalar.copy(
                                    chm_bf[:, k, h * half:(h + 1) * half], cf[:])
                                cr = stream.tile([P, half], F32R, tag="cr")
                                nc.vector.tensor_copy(cr[:], cf[:])
                                for j in range(xb // 2):
                                    xblk = h * (xb // 2) + j
                                    nc.tensor.matmul(
                                        k_ps[xblk][:], wk_r[:, k],
                                        cr[:, 512 * j:512 * (j + 1)],
                                        start=(k == 0), stop=(k == ck - 1))
                        for i in range(xb):
                            nc.scalar.activation(kf_sb[:, 512 * i:512 * (i + 1)],
                                                 k_ps[i][:], IDENT, bias=bk_sb[:])

                    # chmT transposes: deferred so they fill DMA idle slots
                    # during the first softmax block (M1 needs them later).
                    for k in range(ck):
                        nc.sync.dma_start(chmT_bf[:, :, k, :], chm_bf[:, k],
                                          transpose=True)

                    # --- phase 2 (software-pipelined with V^T):
                    #     softmax(0) | V^T | softmax(b+1) interleaved with
                    #     attend(b) so PE fills gaps while ACT/DVE work ahead.
                    with tc.tile_pool(name="pmain", bufs=3) as pmain, \
                         tc.tile_pool(name="zpool", bufs=1) as zpool, \
                         tc.tile_pool(name="rgbf", bufs=1) as rgbf, \
                         tc.tile_pool(name="opool", bufs=2) as opool, \
                         tc.tile_pool(name="m1pool", bufs=2) as m1pool, \
                         tc.tile_pool(name="psS", bufs=2, space="PSUM") as psS, \
                         tc.tile_pool(name="psA", bufs=2, space="PSUM") as psA, \
                         nc.allow_low_precision(reason="softmax weights in bf16"):

                        def softmax_block(blk):
                            ptb = ptpool.tile([P, 4, xt, P], BF16, tag="ptb",
                                              name=f"ptb{blk}")
                            for ht in range(4):
                                htile = blk * 4 + ht
                                p_sb = pmain.tile([P, xy], BF16, tag="p")
                                for s in range(xy // 1024):
                                    s_ps = psS.tile([P, 1024], F32, tag="sps")
                                    # two K=64 matmuls packed into disjoint PE
                                    # row groups run concurrently in the array
                                    nc.tensor.matmul(
                                        s_ps[:, 0:512],
                                        qt_sb[0:cqk, P * htile:P * (htile + 1)],
                                        kf_sb[0:cqk, 1024 * s:1024 * s + 512],
                                        start=True, stop=True,
                                        tile_position=(0, 0))
                                    nc.tensor.matmul(
                                        s_ps[:, 512:1024],
                                        qt_sb[cqk:2 * cqk, P * htile:P * (htile + 1)],
                                        kf_sb[cqk:2 * cqk, 1024 * s + 512:1024 * (s + 1)],
                                        start=True, stop=True,
                                        tile_position=(cqk, 0))
                                    nc.scalar.activation(
                                        p_sb[:, 1024 * s:1024 * (s + 1)], s_ps[:], EXP)
                                # Z = sum over y (pairwise tree, bf16)
                                v3 = p_sb[:].rearrange("p (x y) -> p x y", y=y)
                                tcur = v3
                                w = y
                                while w > 1:
                                    w //= 2
                                    tnext = zpool.tile([P, x_per_tile, w], BF16,
                                                       tag=f"z{w}")
                                    nc.vector.tensor_tensor(
                                        tnext[:], tcur[:, :, 0:w], tcur[:, :, w:2 * w],
                                        ADD)
                                    tcur = tnext
                                rz = zpool.tile([P, x_per_tile, 1], BF16, tag="rz")
                                nc.vector.reciprocal(rz[:], tcur[:])
                                nc.vector.tensor_tensor(
                                    v3, v3, rz[:].to_broadcast([P, x_per_tile, y]),
                                    MULT)
                                nc.sync.dma_start(ptb[:, ht], p_sb[:], transpose=True)
                            return ptb

                        def attend_block(blk, ptb):
                            rg = rgbf.tile([P, ck, 512], F32, tag="rg")
                            ld.dma_start(rg[:],
                                         rgb_t[:, :, 512 * blk:512 * (blk + 1)])
                            # M1[cin, hw] = sum_xy chm[cin, xy] P^T[xy, hw]
                            m1_sb = m1pool.tile([P, ck, 512], BF16, tag="m1")
                            for ch in range(ck):
                                m_ps = psA.tile([P, 512], F32, tag="aps")
                                for m in range(xt):
                                    nc.tensor.matmul(
                                        m_ps[:], chmT_bf[:, m, ch, :],
                                        ptb[:, :, m, :],
                                        start=(m == 0), stop=(m == xt - 1))
                                nc.vector.tensor_copy(m1_sb[:, ch], m_ps[:])
                            # att[c, hw] = sum_cin (gamma Wv)[c, cin] M1[cin, hw]
                            o_sb = opool.tile([P, ck, 512], F32, tag="o")
                            for ct in range(ck):
                                a_ps = psA.tile([P, 512], F32, tag="aps")
                                for ch in range(ck):
                                    nc.tensor.matmul(
                                        a_ps[:], wv_b[:, ch, P * ct:P * (ct + 1)],
                                        m1_sb[:, ch],
                                        start=(ch == 0), stop=(ch == ck - 1))
                                nc.vector.tensor_tensor(o_sb[:, ct], a_ps[:],
                                                        rg[:, ct], ADD)
                            st.dma_start(out_t[:, :, 512 * blk:512 * (blk + 1)],
                                         o_sb[:])

                        ptbs = {0: softmax_block(0)}

                        for blk in range(1, nb):
                            ptbs[blk] = softmax_block(blk)
                            attend_block(blk - 1, ptbs.pop(blk - 1))
                        attend_block(nb - 1, ptbs.pop(nb - 1))

    nc.compile()
    return nc


_NC_CACHE = {}


def _get_nc():
    if "nc" not in _NC_CACHE:
        _NC_CACHE["nc"] = build_program()
    return _NC_CACHE["nc"]


def make_in_maps(rgb_features, chm_features, Wq, bq, Wk, bk, Wv, bv, gamma):
    rgb_features = np.asarray(rgb_features, dtype=np.float32)
    chm_features = np.asarray(chm_features, dtype=np.float32)
    Wq = np.asarray(Wq, dtype=np.float32)
    Wk = np.asarray(Wk, dtype=np.float32)
    Wv = np.asarray(Wv, dtype=np.float32)
    bq = np.asarray(bq, dtype=np.float32)
    bk = np.asarray(bk, dtype=np.float32)
    bv = np.asarray(bv, dtype=np.float32)
    g = float(np.asarray(gamma).reshape(-1)[0])

    wqT = np.ascontiguousarray(np.concatenate([Wq.T, Wq.T], axis=1))
    wkT = np.ascontiguousarray(np.concatenate([Wk.T, Wk.T], axis=1))
    wvT = np.ascontiguousarray((g * Wv).T)
    # softmax rows sum to 1 per (hw, x); summing over the 64 x's makes the
    # bias term contribute exactly 64*gamma*bv[c] to every output pixel.
    rgb_adj = rgb_features + (64.0 * g * bv)[None, :, None, None]
    bq2 = np.ascontiguousarray(np.concatenate([bq, bq]).reshape(2 * CQK, 1))
    bk2 = np.ascontiguousarray(np.concatenate([bk, bk]).reshape(2 * CQK, 1))

    in_maps = []
    for core in range(N_CORES):
        b, half = divmod(core, 2)
        rgb_c = np.ascontiguousarray(
            rgb_adj[b].reshape(C, HW)[:, half * HWC:(half + 1) * HWC])
        chm_c = np.ascontiguousarray(chm_features[b].reshape(C, HW))
        in_maps.append({
            "rgb": rgb_c, "chm": chm_c,
            "wqT": wqT, "wkT": wkT, "wvT": wvT,
            "bq": bq2, "bk": bk2,
        })
    return in_maps


def assemble(results):
    fused = np.empty((B, C, H, W), dtype=np.float32)
    fused2 = fused.reshape(B, C, HW)
    for core in range(N_CORES):
        b, half = divmod(core, 2)
        fused2[b, :, half * HWC:(half + 1) * HWC] = results[core]["out"]
    return fused


def kernel(rgb_features, chm_features, Wq, bq, Wk, bk, Wv, bv, gamma):
    nc = _get_nc()
    in_maps = make_in_maps(rgb_features, chm_features, Wq, bq, Wk, bk, Wv, bv,
                           gamma)
    res = run_bass_kernel_spmd(nc, in_maps, core_ids=list(range(N_CORES)))
    return assemble(res.results)



# revision 24
# speedup vs baseline: 1.0354x; 1.0354x over previous
"""Trainium2 Bass kernel for nn_CrossAttention (B=4, C=512, H=W=64, CQK=64).

Math (per batch b):
    Q = Wq @ rgb + bq                      [CQK, HW]
    K = Wk @ chm + bk                      [CQK, XY]
    S[hw, xy] = sum_o Q[o, hw] K[o, xy]
    P = softmax over y only (xy = x*64 + y)
    att[c, hw] = sum_xy P[hw, xy] V[c, xy],  V = Wv @ chm + bv
    out = rgb + gamma * att

Sharding: 8 cores = 4 batches x 2 halves of the hw (query) axis; weights
replicated; no collectives.

Device dataflow per core (vs the bf16 baseline):
  - S columns are stored y-major ("flip"): column j = y*64 + x, arranged by
    permuting the PSUM->SBUF write APs of K (and chm's fp8 copy). Softmax
    groups (fixed x, all y) are then stride-64 column sets; the Z tree and
    the bf16 normalize multiplies keep stride-1 innermost APs (DVE 2x).
  - M1 = chm @ P^T runs in fp8e4 DoubleRow mode (2 contraction rows per PE
    cell). Both operands are transposed as *uint16 pair views* of fp8
    tiles through the DMA xbar (fp8 alone is not transposable); the pair
    (x=2xp, x=2xp+1 | y) indexing is identical on both sides and the
    matmul's [K, 2, N] APs un-interleave pairs with stride-2 fp8 dims.
  - P normalization: 2 of 4 htiles quantize to fp8 directly on DVE; the
    other 2 normalize bf16 in-place (2x mode) and gpsimd does the fp8
    convert, balancing DVE against the otherwise idle Pool engine.
  - Attend lags softmax by TWO blocks so the DoubleRow matmuls
    interleaved into softmax's exp-paced S gaps never wait on P^T.
  - rgb tiles stay resident from phase 1; the rgb add rides an
    identity-f32r matmul accumulated into the att2 PSUM group.
  - All loads are priority-ordered on the ACT HWDGE ring (the per-core
    HBM port serializes them anyway); transposes ride the SP ring.
  - att2 stays bf16; gamma*Wv and 64*gamma*bv are folded on the host.
"""

import numpy as np

import concourse.bass as bass
import concourse.mybir as mybir
import concourse.tile as tile
from concourse import bacc
from concourse.bass_utils import run_bass_kernel_spmd

P = 128
B, C, H, W = 4, 512, 64, 64
HW = H * W                # 4096
CQK = C // 8              # 64
N_CORES = 8
HWC = HW // 2             # hw rows per core (2048)

F32 = mybir.dt.float32
F32R = mybir.dt.float32r
BF16 = mybir.dt.bfloat16
FP8 = mybir.dt.float8e4
U16 = mybir.dt.uint16
ADD = mybir.AluOpType.add
MULT = mybir.AluOpType.mult
IDENT = mybir.ActivationFunctionType.Identity
EXP = mybir.ActivationFunctionType.Exp
DR = mybir.MatmulPerfMode.DoubleRow


def build_program(hwc=HWC, xy=HW, c=C, cqk=CQK, n_cores=N_CORES):
    """Build the per-core Bass program. Returns a compiled Bacc module."""
    ck = c // P               # channel chunks (4)
    nb = hwc // 512           # hw blocks (4)
    y = 64                    # softmax group size
    nx = xy // y              # x values (64)
    jt = xy // 2 // P         # u16-pair transpose tiles over xy (16)

    nc = bacc.Bacc("TRN2", target_bir_lowering=False, debug=False,
                   num_devices=n_cores)

    rgb = nc.dram_tensor("rgb", [c, hwc], F32, kind="ExternalInput")
    chm = nc.dram_tensor("chm", [c, xy], F32, kind="ExternalInput")
    wqT = nc.dram_tensor("wqT", [c, 2 * cqk], F32, kind="ExternalInput")
    wkT = nc.dram_tensor("wkT", [c, 2 * cqk], F32, kind="ExternalInput")
    wvT = nc.dram_tensor("wvT", [c, c], F32, kind="ExternalInput")
    bq = nc.dram_tensor("bq", [2 * cqk, 1], F32, kind="ExternalInput")
    bk = nc.dram_tensor("bk", [2 * cqk, 1], F32, kind="ExternalInput")
    ident = nc.dram_tensor("ident", [P, P], F32, kind="ExternalInput")
    out = nc.dram_tensor("out", [c, hwc], F32, kind="ExternalOutput")

    rgb_t = rgb.ap().rearrange("(k p) n -> p k n", p=P)
    chm_t = chm.ap().rearrange("(k p) n -> p k n", p=P)
    wq_t = wqT.ap().rearrange("(k p) m -> p k m", p=P)
    wk_t = wkT.ap().rearrange("(k p) m -> p k m", p=P)
    wv_t = wvT.ap().rearrange("(k p) m -> p k m", p=P)
    out_t = out.ap().rearrange("(k p) n -> p k n", p=P)

    with tile.TileContext(nc) as tc, \
         nc.allow_low_precision(reason="attention weights in fp8"):
        with tc.tile_pool(name="persist", bufs=1) as pers:
            # --- persistent tiles ---
            wv_b = pers.tile([P, ck, c], BF16)
            bq_sb = pers.tile([2 * cqk, 1], F32)
            bk_sb = pers.tile([2 * cqk, 1], F32)
            qt_sb = pers.tile([2 * cqk, hwc], BF16)
            kf_sb = pers.tile([2 * cqk, xy], BF16)    # y-major columns
            wq_b = pers.tile([P, ck, 2 * cqk], BF16)
            wk_b = pers.tile([P, ck, 2 * cqk], BF16)
            id_b = pers.tile([P, P], BF16)
            rgb_b = pers.tile([P, ck, hwc], BF16)
            chmT = pers.tile([P, ck, 2 * jt, P], BF16)  # chm^T, bf16

            # --- phase 1 ---
            with tc.tile_pool(name="qload", bufs=2) as qload, \
                 tc.tile_pool(name="kload", bufs=2) as kload, \
                 tc.tile_pool(name="wload", bufs=1) as wload:
                wq_sb = wload.tile([P, ck, 2 * cqk], F32, name="wq_sb")
                wk_sb = wload.tile([P, ck, 2 * cqk], F32, name="wk_sb")
                id_sb = wload.tile([P, P], F32, name="id_sb")
                nc.scalar.dma_start(wq_sb[:], wq_t)
                nc.scalar.dma_start(wk_sb[:], wk_t)
                nc.scalar.dma_start(bq_sb[:], bq.ap())
                nc.scalar.dma_start(bk_sb[:], bk.ap())
                rfh = [qload.tile([P, 2, hwc], F32, tag="rfh",
                                  name=f"rfh{h}") for h in range(2)]
                cfh = [kload.tile([P, 2, xy], F32, tag="cfh",
                                  name=f"cfh{h}") for h in range(2)]
                cb = {}
                cf = [cfh[k // 2][:, k % 2] for k in range(ck)]

                nc.scalar.dma_start(rfh[0][:], rgb_t[:, 0:2])
                nc.scalar.dma_start(rfh[1][:], rgb_t[:, 2:4])
                nc.scalar.dma_start(cfh[0][:], chm_t[:, 0:2])
                nc.scalar.dma_start(cfh[1][:], chm_t[:, 2:4])
                wv_f = wload.tile([P, ck, c], F32)
                nc.scalar.dma_start(wv_f[:], wv_t)
                nc.scalar.dma_start(id_sb[:], ident.ap())
                nc.gpsimd.tensor_copy(wv_b[:], wv_f[:])
                nc.vector.tensor_copy(wq_b[:], wq_sb[:])
                nc.vector.tensor_copy(wk_b[:], wk_sb[:])
                nc.vector.tensor_copy(id_b[:], id_sb[:])
                for h in range(2):
                    # bf16 copy of rgb (DVE idle in phase 1)
                    nc.vector.tensor_copy(rgb_b[:, 2 * h:2 * h + 2],
                                          rfh[h][:])

                for k in range(ck):
                    # bf16 copies of chm chunk halves: K GEMM rhs + the
                    # chm^T transpose source (SP ring, as soon as ready)
                    cfv = cf[k][:].rearrange("p (xx yy) -> p yy xx", yy=y)
                    for h in range(2):
                        cb[k, h] = kload.tile([P, 32, nx], BF16, tag="cb",
                                              name=f"cb{k}_{h}")
                        nc.vector.tensor_copy(
                            cb[k, h][:], cfv[:, 32 * h:32 * (h + 1)])
                        nc.sync.dma_start(chmT[:, k, 16 * h:16 * (h + 1)],
                                          cb[k, h][:], transpose=True)

                with tc.tile_pool(name="psQ", bufs=2, space="PSUM") as psQ:
                    # Q j-outer: one bank per 512-col block, 2 banks rotate
                    for j in range(nb):
                        q_ps = psQ.tile([2 * cqk, 512], F32, tag="qps",
                                        name="q_ps")
                        for k in range(ck):
                            nc.tensor.matmul(
                                q_ps[:], wq_b[:, k],
                                rgb_b[:, k, 512 * j:512 * (j + 1)],
                                start=(k == 0), stop=(k == ck - 1))
                        nc.scalar.activation(
                            qt_sb[:, 512 * j:512 * (j + 1)], q_ps[:],
                            IDENT, bias=bq_sb[:])

                with tc.tile_pool(name="psK", bufs=1, space="PSUM") as psK:
                    # K full-width k-outer in bf16 (8 banks), trailing the
                    # chm chunk loads + bf16 converts.
                    k_ps = [psK.tile([2 * cqk, 2048], F32, name=f"kps{i}")
                            for i in range(2)]
                    for k in range(ck):
                        for j in range(8):
                            nc.tensor.matmul(
                                k_ps[j // 4][:, 512 * (j % 4):
                                             512 * (j % 4 + 1)],
                                wk_b[:, k],
                                cb[k, j // 4][:]
                                .rearrange("p a b -> p (a b)")
                                [:, 512 * (j % 4):512 * (j % 4 + 1)],
                                start=(k == 0), stop=(k == ck - 1))
                    for i in range(2):
                        nc.scalar.activation(
                            kf_sb[:, 2048 * i:2048 * (i + 1)], k_ps[i][:],
                            IDENT, bias=bk_sb[:])

            # --- phase 2: softmax(b) / attend(b-2) pipeline ---
            with tc.tile_pool(name="pmain", bufs=3) as pmain, \
                 tc.tile_pool(name="ptpool", bufs=2) as ptpool, \
                 tc.tile_pool(name="zpool", bufs=1) as zpool, \
                 tc.tile_pool(name="opool", bufs=1) as opool, \
                 tc.tile_pool(name="m1pool", bufs=2) as m1pool, \
                 tc.tile_pool(name="psS", bufs=2, space="PSUM") as psS, \
                 tc.tile_pool(name="psA", bufs=2, space="PSUM") as psA:

                ptb = {}
                m1t = {}

                def m1_quarter(blk, ch, quarter):
                    """8 bf16 matmuls: one quarter of M1[ch, blk]."""
                    if blk not in m1t:
                        m1t[blk] = m1pool.tile([P, ck, 512], BF16, tag="m1",
                                               name=f"m1_{blk}")
                    if ch % 2 == 0 and quarter == 0:
                        m1t[(blk, "ps", ch // 2)] = psA.tile(
                            [P, 1024], F32, tag="aps", name="m1ps")
                    m_ps = m1t[(blk, "ps", ch // 2)]
                    half = m_ps[:, 512 * (ch % 2):512 * (ch % 2 + 1)]
                    for m in range(8 * quarter, 8 * quarter + 8):
                        nc.tensor.matmul(
                            half, chmT[:, ch, m],
                            ptb[blk][:, m].rearrange("p hh nn -> p (hh nn)"),
                            start=(m == 0), stop=(m == 2 * jt - 1))
                    if ch % 2 == 1 and quarter == 3:
                        m_ps = m1t.pop((blk, "ps", ch // 2))
                        nc.scalar.activation(
                            m1t[blk][:, ch - 1:ch + 1], m_ps[:]
                            .rearrange("p (a b) -> p a b", a=2), IDENT)

                def softmax_htile(blk, ht, interleave, pool_convert):
                    """S matmuls + exp + normalize + fp8 transpose for one
                    128-row hw tile; S-chunk matmuls alternate with
                    `interleave` chunks (callables emitting PE work)."""
                    htile = blk * 4 + ht
                    p_sb = pmain.tile([P, xy], BF16, tag="p")
                    for s in range(xy // 1024):
                        s_ps = psS.tile([P, 1024], F32, tag="sps")
                        nc.tensor.matmul(
                            s_ps[:, 0:512],
                            qt_sb[0:cqk, P * htile:P * (htile + 1)],
                            kf_sb[0:cqk, 1024 * s:1024 * s + 512],
                            start=True, stop=True, tile_position=(0, 0))
                        nc.tensor.matmul(
                            s_ps[:, 512:1024],
                            qt_sb[cqk:2 * cqk, P * htile:P * (htile + 1)],
                            kf_sb[cqk:2 * cqk, 1024 * s + 512:1024 * (s + 1)],
                            start=True, stop=True, tile_position=(cqk, 0))
                        nc.scalar.activation(
                            p_sb[:, 1024 * s:1024 * (s + 1)], s_ps[:], EXP)
                        if s < len(interleave):
                            interleave[s]()
                    # Z[hw, x] = sum_y P_u[hw, y, x]: pairwise tree over the
                    # y (outer) dim keeps stride-1 innermost (DVE 2x).
                    v3 = p_sb[:].rearrange("p (yy xx) -> p yy xx", xx=nx)
                    tcur = v3
                    w = y
                    while w > 1:
                        w //= 2
                        tnext = zpool.tile([P, w, nx], BF16, tag=f"z{w}")
                        nc.vector.tensor_tensor(
                            tnext[:], tcur[:, 0:w], tcur[:, w:2 * w], ADD)
                        tcur = tnext
                    rz = zpool.tile([P, 1, nx], BF16, tag="rz")
                    nc.vector.reciprocal(rz[:], tcur[:])
                    # normalize in place at DVE 2x (the y-major layout keeps
                    # every innermost stride 1)
                    nc.vector.tensor_tensor(
                        v3, v3, rz[:].to_broadcast([P, y, nx]), MULT)
                    if blk not in ptb:
                        # [p, m, ht, hw]: hw contiguous across ht per m so
                        # the M1 rhs merges to one 512-wide moving AP.
                        ptb[blk] = ptpool.tile([P, 2 * jt, 4, P], BF16,
                                               tag="ptb", name=f"ptb{blk}")
                    nc.sync.dma_start(ptb[blk][:, :, ht], p_sb[:],
                                      transpose=True)

                def attend_fini(blk):
                    """att2 + rgb add (via identity matmul) + store."""
                    m1_sb = m1t.pop(blk)
                    o_sb = opool.tile([P, ck, 512], F32, tag="o")
                    for cp in range(2):
                        a_ps = psA.tile([P, 1024], F32, tag="aps",
                                        name="a_ps")
                        for h in range(2):
                            ct = 2 * cp + h
                            half = a_ps[:, 512 * h:512 * (h + 1)]
                            for chh in range(ck):
                                nc.tensor.matmul(
                                    half, wv_b[:, chh, P * ct:P * (ct + 1)],
                                    m1_sb[:, chh],
                                    start=(chh == 0), stop=False)
                            nc.tensor.matmul(
                                half, id_b[:],
                                rgb_b[:, ct, 512 * blk:512 * (blk + 1)],
                                start=False, stop=True)
                        nc.scalar.activation(
                            o_sb[:, 2 * cp:2 * cp + 2],
                            a_ps[:].rearrange("p (a b) -> p a b", a=2), IDENT)
                    nc.sync.dma_start(
                        out_t[:, :, 512 * blk:512 * (blk + 1)], o_sb[:])
                    del ptb[blk]

                def slot(ab, ch):
                    return [(lambda c=ch, q=q: m1_quarter(ab, c, q))
                            for q in range(4)]

                # depth-1.5 pipeline: softmax(b) hosts the second half of
                # attend(b-2), its fini, then the first half of attend(b-1)
                for blk in range(nb):
                    for ht in range(4):
                        ab, ch = blk - 2 + ht // 2, (ht % 2) + 2 * (ht // 2 == 0)
                        ivl = slot(ab, ch) if 0 <= ab < nb else []
                        softmax_htile(blk, ht, ivl, pool_convert=ht % 2 == 1)
                        if blk >= 2 and ht == 1:
                            attend_fini(blk - 2)
                # tail: finish attend(nb-2) and attend(nb-1)
                for ch in range(2, ck):
                    for q in range(4):
                        m1_quarter(nb - 2, ch, q)
                attend_fini(nb - 2)
                for ch in range(ck):
                    for q in range(4):
                        m1_quarter(nb - 1, ch, q)
                attend_fini(nb - 1)

    nc.compile()
    return nc


_NC_CACHE = {}


def _get_nc():
    if "nc" not in _NC_CACHE:
        _NC_CACHE["nc"] = build_program()
    return _NC_CACHE["nc"]


def make_in_maps(rgb_features, chm_features, Wq, bq, Wk, bk, Wv, bv, gamma):
    rgb_features = np.asarray(rgb_features, dtype=np.float32)
    chm_features = np.asarray(chm_features, dtype=np.float32)
    Wq = np.asarray(Wq, dtype=np.float32)
    Wk = np.asarray(Wk, dtype=np.float32)
    Wv = np.asarray(Wv, dtype=np.float32)
    bq = np.asarray(bq, dtype=np.float32)
    bk = np.asarray(bk, dtype=np.float32)
    bv = np.asarray(bv, dtype=np.float32)
    g = float(np.asarray(gamma).reshape(-1)[0])

    wqT = np.ascontiguousarray(np.concatenate([Wq.T, Wq.T], axis=1))
    wkT = np.ascontiguousarray(np.concatenate([Wk.T, Wk.T], axis=1))
    wvT = np.ascontiguousarray((g * Wv).T)
    # softmax rows sum to 1 per (hw, x); summing over the 64 x's makes the
    # bias term contribute exactly 64*gamma*bv[c] to every output pixel.
    rgb_adj = rgb_features + (64.0 * g * bv)[None, :, None, None]
    bq2 = np.ascontiguousarray(np.concatenate([bq, bq]).reshape(2 * CQK, 1))
    bk2 = np.ascontiguousarray(np.concatenate([bk, bk]).reshape(2 * CQK, 1))
    ident = np.eye(P, dtype=np.float32)

    in_maps = []
    for core in range(N_CORES):
        b, half = divmod(core, 2)
        rgb_c = np.ascontiguousarray(
            rgb_adj[b].reshape(C, HW)[:, half * HWC:(half + 1) * HWC])
        chm_c = np.ascontiguousarray(chm_features[b].reshape(C, HW))
        in_maps.append({
            "rgb": rgb_c, "chm": chm_c,
            "wqT": wqT, "wkT": wkT, "wvT": wvT,
            "bq": bq2, "bk": bk2, "ident": ident,
        })
    return in_maps


def assemble(results):
    fused = np.empty((B, C, H, W), dtype=np.float32)
    fused2 = fused.reshape(B, C, HW)
    for core in range(N_CORES):
        b, half = divmod(core, 2)
        fused2[b, :, half * HWC:(half + 1) * HWC] = results[core]["out"]
    return fused


def kernel(rgb_features, chm_features, Wq, bq, Wk, bk, Wv, bv, gamma):
    nc = _get_nc()
    in_maps = make_in_maps(rgb_features, chm_features, Wq, bq, Wk, bk, Wv, bv,
                           gamma)
    res = run_bass_kernel_spmd(nc, in_maps, core_ids=list(range(N_CORES)))
    return assemble(res.results)


# revision 25
# speedup vs baseline: 1.1400x; 1.1009x over previous
"""Trainium2 Bass kernel for nn_CrossAttention (B=4, C=512, H=W=64, CQK=64).

Math (per batch b):
    Q = Wq @ rgb + bq                      [CQK, HW]
    K = Wk @ chm + bk                      [CQK, XY]
    S[hw, xy] = sum_o Q[o, hw] K[o, xy]
    P = softmax over y only (xy = x*64 + y)
    att[c, hw] = sum_xy P[hw, xy] V[c, xy],  V = Wv @ chm + bv
    out = rgb + gamma * att

Sharding: 8 cores = 4 batches x 2 halves of the hw (query) axis; weights
replicated; no collectives.

Device dataflow per core (vs the bf16 baseline):
  - S columns are stored y-major ("flip"): column j = y*64 + x, arranged by
    permuting the PSUM->SBUF write APs of K (and chm's fp8 copy). Softmax
    groups (fixed x, all y) are then stride-64 column sets; the Z tree and
    the bf16 normalize multiplies keep stride-1 innermost APs (DVE 2x).
  - M1 = chm @ P^T runs in fp8e4 DoubleRow mode (2 contraction rows per PE
    cell). Both operands are transposed as *uint16 pair views* of fp8
    tiles through the DMA xbar (fp8 alone is not transposable); the pair
    (x=2xp, x=2xp+1 | y) indexing is identical on both sides and the
    matmul's [K, 2, N] APs un-interleave pairs with stride-2 fp8 dims.
  - P normalization: 2 of 4 htiles quantize to fp8 directly on DVE; the
    other 2 normalize bf16 in-place (2x mode) and gpsimd does the fp8
    convert, balancing DVE against the otherwise idle Pool engine.
  - Attend lags softmax by TWO blocks so the DoubleRow matmuls
    interleaved into softmax's exp-paced S gaps never wait on P^T.
  - rgb tiles stay resident from phase 1; the rgb add rides an
    identity-f32r matmul accumulated into the att2 PSUM group.
  - All loads are priority-ordered on the ACT HWDGE ring (the per-core
    HBM port serializes them anyway); transposes ride the SP ring.
  - att2 stays bf16; gamma*Wv and 64*gamma*bv are folded on the host.
"""

import numpy as np

import concourse.bass as bass
import concourse.mybir as mybir
import concourse.tile as tile
from concourse import bacc
from concourse.bass_utils import run_bass_kernel_spmd

P = 128
B, C, H, W = 4, 512, 64, 64
HW = H * W                # 4096
CQK = C // 8              # 64
N_CORES = 8
HWC = HW // 2             # hw rows per core (2048)

F32 = mybir.dt.float32
F32R = mybir.dt.float32r
BF16 = mybir.dt.bfloat16
FP8 = mybir.dt.float8e4
U16 = mybir.dt.uint16
ADD = mybir.AluOpType.add
MULT = mybir.AluOpType.mult
IDENT = mybir.ActivationFunctionType.Identity
EXP = mybir.ActivationFunctionType.Exp
DR = mybir.MatmulPerfMode.DoubleRow


def build_program(hwc=HWC, xy=HW, c=C, cqk=CQK, n_cores=N_CORES):
    """Build the per-core Bass program. Returns a compiled Bacc module."""
    ck = c // P               # channel chunks (4)
    nb = hwc // 512           # hw blocks (4)
    y = 64                    # softmax group size
    nx = xy // y              # x values (64)
    jt = xy // 2 // P         # u16-pair transpose tiles over xy (16)

    nc = bacc.Bacc("TRN2", target_bir_lowering=False, debug=False,
                   num_devices=n_cores)

    rgb = nc.dram_tensor("rgb", [c, hwc], F32, kind="ExternalInput")
    chm = nc.dram_tensor("chm", [c, xy], F32, kind="ExternalInput")
    wqT = nc.dram_tensor("wqT", [c, 2 * cqk], F32, kind="ExternalInput")
    wkT = nc.dram_tensor("wkT", [c, 2 * cqk], F32, kind="ExternalInput")
    wvT = nc.dram_tensor("wvT", [c, c], F32, kind="ExternalInput")
    bq = nc.dram_tensor("bq", [2 * cqk, 1], F32, kind="ExternalInput")
    bk = nc.dram_tensor("bk", [2 * cqk, 1], F32, kind="ExternalInput")
    ident = nc.dram_tensor("ident", [P, P], F32, kind="ExternalInput")
    out = nc.dram_tensor("out", [c, hwc], F32, kind="ExternalOutput")

    rgb_t = rgb.ap().rearrange("(k p) n -> p k n", p=P)
    chm_t = chm.ap().rearrange("(k p) n -> p k n", p=P)
    wq_t = wqT.ap().rearrange("(k p) m -> p k m", p=P)
    wk_t = wkT.ap().rearrange("(k p) m -> p k m", p=P)
    wv_t = wvT.ap().rearrange("(k p) m -> p k m", p=P)
    out_t = out.ap().rearrange("(k p) n -> p k n", p=P)

    with tile.TileContext(nc) as tc, \
         nc.allow_low_precision(reason="attention weights in fp8"):
        with tc.tile_pool(name="persist", bufs=1) as pers:
            # --- persistent tiles ---
            wv_b = pers.tile([P, ck, c], BF16)
            bq_sb = pers.tile([2 * cqk, 1], F32)
            bk_sb = pers.tile([2 * cqk, 1], F32)
            qt_sb = pers.tile([2 * cqk, hwc], BF16)
            kf_sb = pers.tile([2 * cqk, xy], BF16)    # y-major columns
            wq_b = pers.tile([P, ck, 2 * cqk], BF16)
            wk_b = pers.tile([P, ck, 2 * cqk], BF16)
            id_b = pers.tile([P, P], BF16)
            rgb_b = pers.tile([P, ck, hwc], BF16)
            chmT8c = pers.tile([P, ck, jt, 2, P], FP8)  # chm^T, canonical

            # --- phase 1 ---
            with tc.tile_pool(name="qload", bufs=2) as qload, \
                 tc.tile_pool(name="kload", bufs=2) as kload, \
                 tc.tile_pool(name="wload", bufs=1) as wload, \
                 tc.tile_pool(name="c8pool", bufs=2) as c8pool, \
                 tc.tile_pool(name="ctpool", bufs=2) as ctpool:
                wq_sb = wload.tile([P, ck, 2 * cqk], F32, name="wq_sb")
                wk_sb = wload.tile([P, ck, 2 * cqk], F32, name="wk_sb")
                id_sb = wload.tile([P, P], F32, name="id_sb")
                nc.scalar.dma_start(wq_sb[:], wq_t)
                nc.scalar.dma_start(wk_sb[:], wk_t)
                nc.scalar.dma_start(bq_sb[:], bq.ap())
                nc.scalar.dma_start(bk_sb[:], bk.ap())
                rfh = [qload.tile([P, 2, hwc], F32, tag="rfh",
                                  name=f"rfh{h}") for h in range(2)]
                cfh = [kload.tile([P, 2, xy], F32, tag="cfh",
                                  name=f"cfh{h}") for h in range(2)]
                cb = {}
                cf = [cfh[k // 2][:, k % 2] for k in range(ck)]

                nc.scalar.dma_start(rfh[0][:], rgb_t[:, 0:2])
                nc.scalar.dma_start(rfh[1][:], rgb_t[:, 2:4])
                nc.scalar.dma_start(cfh[0][:], chm_t[:, 0:2])
                nc.scalar.dma_start(cfh[1][:], chm_t[:, 2:4])
                wv_f = wload.tile([P, ck, c], F32)
                nc.scalar.dma_start(wv_f[:], wv_t)
                nc.scalar.dma_start(id_sb[:], ident.ap())
                nc.gpsimd.tensor_copy(wv_b[:], wv_f[:])
                nc.vector.tensor_copy(wq_b[:], wq_sb[:])
                nc.vector.tensor_copy(wk_b[:], wk_sb[:])
                nc.vector.tensor_copy(id_b[:], id_sb[:])
                for h in range(2):
                    # bf16 copy of rgb (DVE idle in phase 1)
                    nc.vector.tensor_copy(rgb_b[:, 2 * h:2 * h + 2],
                                          rfh[h][:])

                for k in range(ck):
                    # bf16 copies of chm chunk halves: K GEMM rhs + the
                    # chm^T transpose source (SP ring, as soon as ready)
                    cfv = cf[k][:].rearrange("p (xx yy) -> p yy xx", yy=y)
                    for h in range(2):
                        cb[k, h] = kload.tile([P, 32, nx], BF16, tag="cb",
                                              name=f"cb{k}_{h}")
                        nc.vector.tensor_copy(
                            cb[k, h][:], cfv[:, 32 * h:32 * (h + 1)])
                    # fp8 permuted copy (Pool), u16-pair transpose, then
                    # de-interleave to the canonical [K, s, M] weights
                    # layout on ACT (walrus rejects strided DR weights)
                    chm8k = c8pool.tile([P, xy], FP8, tag="chm8",
                                        name=f"chm8_{k}")
                    c8v = chm8k[:].rearrange("p (yy xx) -> p xx yy", xx=nx)
                    nc.gpsimd.tensor_copy(
                        c8v, cf[k][:].rearrange("p (xx yy) -> p xx yy",
                                                yy=y))
                    ct8 = ctpool.tile([P, jt, P], U16, tag="ct8",
                                      name=f"ct8_{k}")
                    nc.sync.dma_start(ct8[:], chm8k[:].bitcast(U16),
                                      transpose=True)
                    nc.scalar.activation(
                        chmT8c[:, k],
                        ct8[:].bitcast(FP8)
                        .rearrange("p m (cc ss) -> p m ss cc", ss=2),
                        IDENT)

                with tc.tile_pool(name="psQ", bufs=2, space="PSUM") as psQ:
                    # Q j-outer: one bank per 512-col block, 2 banks rotate
                    for j in range(nb):
                        q_ps = psQ.tile([2 * cqk, 512], F32, tag="qps",
                                        name="q_ps")
                        for k in range(ck):
                            nc.tensor.matmul(
                                q_ps[:], wq_b[:, k],
                                rgb_b[:, k, 512 * j:512 * (j + 1)],
                                start=(k == 0), stop=(k == ck - 1))
                        nc.scalar.activation(
                            qt_sb[:, 512 * j:512 * (j + 1)], q_ps[:],
                            IDENT, bias=bq_sb[:])

                with tc.tile_pool(name="psK", bufs=1, space="PSUM") as psK:
                    # K full-width k-outer in bf16 (8 banks), trailing the
                    # chm chunk loads + bf16 converts.
                    k_ps = [psK.tile([2 * cqk, 2048], F32, name=f"kps{i}")
                            for i in range(2)]
                    for k in range(ck):
                        for j in range(8):
                            nc.tensor.matmul(
                                k_ps[j // 4][:, 512 * (j % 4):
                                             512 * (j % 4 + 1)],
                                wk_b[:, k],
                                cb[k, j // 4][:]
                                .rearrange("p a b -> p (a b)")
                                [:, 512 * (j % 4):512 * (j % 4 + 1)],
                                start=(k == 0), stop=(k == ck - 1))
                    for i in range(2):
                        nc.scalar.activation(
                            kf_sb[:, 2048 * i:2048 * (i + 1)], k_ps[i][:],
                            IDENT, bias=bk_sb[:])

            # --- phase 2: softmax(b) / attend(b-2) pipeline ---
            with tc.tile_pool(name="pmain", bufs=3) as pmain, \
                 tc.tile_pool(name="p8pool", bufs=2) as p8pool, \
                 tc.tile_pool(name="ptpool", bufs=2) as ptpool, \
                 tc.tile_pool(name="zpool", bufs=1) as zpool, \
                 tc.tile_pool(name="opool", bufs=1) as opool, \
                 tc.tile_pool(name="m1pool", bufs=2) as m1pool, \
                 tc.tile_pool(name="psS", bufs=2, space="PSUM") as psS, \
                 tc.tile_pool(name="psA", bufs=2, space="PSUM") as psA:

                ptb = {}
                m1t = {}

                def m1_quarter(blk, ch, quarter):
                    """4 DoubleRow matmuls: one quarter of M1[ch, blk]."""
                    if blk not in m1t:
                        m1t[blk] = m1pool.tile([P, ck, 512], BF16, tag="m1",
                                               name=f"m1_{blk}")
                    if ch % 2 == 0 and quarter == 0:
                        m1t[(blk, "ps", ch // 2)] = psA.tile(
                            [P, 1024], F32, tag="aps", name="m1ps")
                    m_ps = m1t[(blk, "ps", ch // 2)]
                    half = m_ps[:, 512 * (ch % 2):512 * (ch % 2 + 1)]
                    for m in range(4 * quarter, 4 * quarter + 4):
                        rhs = (ptb[blk][:, m]
                               .rearrange("p hh nn -> p (hh nn)")
                               .bitcast(FP8)
                               .rearrange("p (nn ss) -> p ss nn", ss=2))
                        nc.tensor.matmul(
                            half, chmT8c[:, ch, m], rhs, perf_mode=DR,
                            start=(m == 0), stop=(m == jt - 1))
                    if ch % 2 == 1 and quarter == 3:
                        m_ps = m1t.pop((blk, "ps", ch // 2))
                        nc.scalar.activation(
                            m1t[blk][:, ch - 1:ch + 1], m_ps[:]
                            .rearrange("p (a b) -> p a b", a=2), IDENT)

                def softmax_htile(blk, ht, interleave, pool_convert):
                    """S matmuls + exp + normalize + fp8 transpose for one
                    128-row hw tile; S-chunk matmuls alternate with
                    `interleave` chunks (callables emitting PE work)."""
                    htile = blk * 4 + ht
                    p_sb = pmain.tile([P, xy], BF16, tag="p")
                    for s in range(xy // 1024):
                        s_ps = psS.tile([P, 1024], F32, tag="sps")
                        nc.tensor.matmul(
                            s_ps[:, 0:512],
                            qt_sb[0:cqk, P * htile:P * (htile + 1)],
                            kf_sb[0:cqk, 1024 * s:1024 * s + 512],
                            start=True, stop=True, tile_position=(0, 0))
                        nc.tensor.matmul(
                            s_ps[:, 512:1024],
                            qt_sb[cqk:2 * cqk, P * htile:P * (htile + 1)],
                            kf_sb[cqk:2 * cqk, 1024 * s + 512:1024 * (s + 1)],
                            start=True, stop=True, tile_position=(cqk, 0))
                        nc.scalar.activation(
                            p_sb[:, 1024 * s:1024 * (s + 1)], s_ps[:], EXP)
                        if s < len(interleave):
                            interleave[s]()
                    # Z[hw, x] = sum_y P_u[hw, y, x]: pairwise tree over the
                    # y (outer) dim keeps stride-1 innermost (DVE 2x).
                    v3 = p_sb[:].rearrange("p (yy xx) -> p yy xx", xx=nx)
                    tcur = v3
                    w = y
                    while w > 1:
                        w //= 2
                        tnext = zpool.tile([P, w, nx], BF16, tag=f"z{w}")
                        nc.vector.tensor_tensor(
                            tnext[:], tcur[:, 0:w], tcur[:, w:2 * w], ADD)
                        tcur = tnext
                    rz = zpool.tile([P, 1, nx], BF16, tag="rz")
                    nc.vector.reciprocal(rz[:], tcur[:])
                    # normalize + quantize to fp8 in one DVE pass
                    p8 = p8pool.tile([P, y, nx], FP8, tag="p8")
                    nc.vector.tensor_tensor(
                        p8[:], v3, rz[:].to_broadcast([P, y, nx]), MULT)
                    if blk not in ptb:
                        # [p, m, ht, hw]: hw contiguous across ht per m so
                        # the M1 rhs merges to one 512-wide moving AP.
                        ptb[blk] = ptpool.tile([P, jt, 4, P], U16,
                                               tag="ptb", name=f"ptb{blk}")
                    nc.sync.dma_start(ptb[blk][:, :, ht], p8[:].bitcast(U16),
                                      transpose=True)

                def attend_fini(blk):
                    """att2 + rgb add (via identity matmul) + store."""
                    m1_sb = m1t.pop(blk)
                    o_sb = opool.tile([P, ck, 512], F32, tag="o")
                    for cp in range(2):
                        a_ps = psA.tile([P, 1024], F32, tag="aps",
                                        name="a_ps")
                        for h in range(2):
                            ct = 2 * cp + h
                            half = a_ps[:, 512 * h:512 * (h + 1)]
                            for chh in range(ck):
                                nc.tensor.matmul(
                                    half, wv_b[:, chh, P * ct:P * (ct + 1)],
                                    m1_sb[:, chh],
                                    start=(chh == 0), stop=False)
                            nc.tensor.matmul(
                                half, id_b[:],
                                rgb_b[:, ct, 512 * blk:512 * (blk + 1)],
                                start=False, stop=True)
                        nc.scalar.activation(
                            o_sb[:, 2 * cp:2 * cp + 2],
                            a_ps[:].rearrange("p (a b) -> p a b", a=2), IDENT)
                    nc.sync.dma_start(
                        out_t[:, :, 512 * blk:512 * (blk + 1)], o_sb[:])
                    del ptb[blk]

                def slot(ab, ch):
                    return [(lambda c=ch, q=q: m1_quarter(ab, c, q))
                            for q in range(4)]

                # depth-1.5 pipeline: softmax(b) hosts the second half of
                # attend(b-2), its fini, then the first half of attend(b-1)
                for blk in range(nb):
                    for ht in range(4):
                        ab, ch = blk - 2 + ht // 2, (ht % 2) + 2 * (ht // 2 == 0)
                        ivl = slot(ab, ch) if 0 <= ab < nb else []
                        softmax_htile(blk, ht, ivl, pool_convert=ht % 2 == 1)
                        if blk >= 2 and ht == 1:
                            attend_fini(blk - 2)
                # tail: finish attend(nb-2) and attend(nb-1)
                for ch in range(2, ck):
                    for q in range(4):
                        m1_quarter(nb - 2, ch, q)
                attend_fini(nb - 2)
                for ch in range(ck):
                    for q in range(4):
                        m1_quarter(nb - 1, ch, q)
                attend_fini(nb - 1)

    nc.compile()
    return nc


_NC_CACHE = {}


def _get_nc():
    if "nc" not in _NC_CACHE:
        _NC_CACHE["nc"] = build_program()
    return _NC_CACHE["nc"]


def make_in_maps(rgb_features, chm_features, Wq, bq, Wk, bk, Wv, bv, gamma):
    rgb_features = np.asarray(rgb_features, dtype=np.float32)
    chm_features = np.asarray(chm_features, dtype=np.float32)
    Wq = np.asarray(Wq, dtype=np.float32)
    Wk = np.asarray(Wk, dtype=np.float32)
    Wv = np.asarray(Wv, dtype=np.float32)
    bq = np.asarray(bq, dtype=np.float32)
    bk = np.asarray(bk, dtype=np.float32)
    bv = np.asarray(bv, dtype=np.float32)
    g = float(np.asarray(gamma).reshape(-1)[0])

    wqT = np.ascontiguousarray(np.concatenate([Wq.T, Wq.T], axis=1))
    wkT = np.ascontiguousarray(np.concatenate([Wk.T, Wk.T], axis=1))
    wvT = np.ascontiguousarray((g * Wv).T)
    # softmax rows sum to 1 per (hw, x); summing over the 64 x's makes the
    # bias term contribute exactly 64*gamma*bv[c] to every output pixel.
    rgb_adj = rgb_features + (64.0 * g * bv)[None, :, None, None]
    bq2 = np.ascontiguousarray(np.concatenate([bq, bq]).reshape(2 * CQK, 1))
    bk2 = np.ascontiguousarray(np.concatenate([bk, bk]).reshape(2 * CQK, 1))
    ident = np.eye(P, dtype=np.float32)

    in_maps = []
    for core in range(N_CORES):
        b, half = divmod(core, 2)
        rgb_c = np.ascontiguousarray(
            rgb_adj[b].reshape(C, HW)[:, half * HWC:(half + 1) * HWC])
        chm_c = np.ascontiguousarray(chm_features[b].reshape(C, HW))
        in_maps.append({
            "rgb": rgb_c, "chm": chm_c,
            "wqT": wqT, "wkT": wkT, "wvT": wvT,
            "bq": bq2, "bk": bk2, "ident": ident,
        })
    return in_maps


def assemble(results):
    fused = np.empty((B, C, H, W), dtype=np.float32)
    fused2 = fused.reshape(B, C, HW)
    for core in range(N_CORES):
        b, half = divmod(core, 2)
        fused2[b, :, half * HWC:(half + 1) * HWC] = results[core]["out"]
    return fused


def kernel(rgb_features, chm_features, Wq, bq, Wk, bk, Wv, bv, gamma):
    nc = _get_nc()
    in_maps = make_in_maps(rgb_features, chm_features, Wq, bq, Wk, bk, Wv, bv,
                           gamma)
    res = run_bass_kernel_spmd(nc, in_maps, core_ids=list(range(N_CORES)))
    return assemble(res.results)


# revision 31
# speedup vs baseline: 1.2509x; 1.0973x over previous
"""Trainium2 Bass kernel for nn_CrossAttention (B=4, C=512, H=W=64, CQK=64).

Math (per batch b):
    Q = Wq @ rgb + bq                      [CQK, HW]
    K = Wk @ chm + bk                      [CQK, XY]
    S[hw, xy] = sum_o Q[o, hw] K[o, xy]
    P = softmax over y only (xy = x*64 + y)
    att[c, hw] = sum_xy P[hw, xy] V[c, xy],  V = Wv @ chm + bv
    out = rgb + gamma * att

Sharding: 8 cores = 4 batches x 2 halves of the hw (query) axis; weights
replicated; no collectives.

Device dataflow per core (vs the bf16 baseline):
  - S columns are stored y-major ("flip"): column j = y*64 + x, arranged by
    permuting the PSUM->SBUF write APs of K (and chm's fp8 copy). Softmax
    groups (fixed x, all y) are then stride-64 column sets; the Z tree and
    the bf16 normalize multiplies keep stride-1 innermost APs (DVE 2x).
  - M1 = chm @ P^T runs in fp8e4 DoubleRow mode (2 contraction rows per PE
    cell). Both operands are transposed as *uint16 pair views* of fp8
    tiles through the DMA xbar (fp8 alone is not transposable); the pair
    (x=2xp, x=2xp+1 | y) indexing is identical on both sides and the
    matmul's [K, 2, N] APs un-interleave pairs with stride-2 fp8 dims.
  - P normalization: 2 of 4 htiles quantize to fp8 directly on DVE; the
    other 2 normalize bf16 in-place (2x mode) and gpsimd does the fp8
    convert, balancing DVE against the otherwise idle Pool engine.
  - Attend lags softmax by TWO blocks so the DoubleRow matmuls
    interleaved into softmax's exp-paced S gaps never wait on P^T.
  - rgb tiles stay resident from phase 1; the rgb add rides an
    identity-f32r matmul accumulated into the att2 PSUM group.
  - All loads are priority-ordered on the ACT HWDGE ring (the per-core
    HBM port serializes them anyway); transposes ride the SP ring.
  - att2 stays bf16; gamma*Wv and 64*gamma*bv are folded on the host.
"""

import numpy as np

import concourse.bass as bass
import concourse.mybir as mybir
import concourse.tile as tile
from concourse import bacc
from concourse.bass_utils import run_bass_kernel_spmd

P = 128
B, C, H, W = 4, 512, 64, 64
HW = H * W                # 4096
CQK = C // 8              # 64
N_CORES = 8
HWC = HW // 2             # hw rows per core (2048)

F32 = mybir.dt.float32
F32R = mybir.dt.float32r
BF16 = mybir.dt.bfloat16
FP8 = mybir.dt.float8e4
U16 = mybir.dt.uint16
ADD = mybir.AluOpType.add
MULT = mybir.AluOpType.mult
IDENT = mybir.ActivationFunctionType.Identity
EXP = mybir.ActivationFunctionType.Exp
DR = mybir.MatmulPerfMode.DoubleRow


def build_program(hwc=HWC, xy=HW, c=C, cqk=CQK, n_cores=N_CORES):
    """Build the per-core Bass program. Returns a compiled Bacc module."""
    ck = c // P               # channel chunks (4)
    nb = hwc // 512           # hw blocks (4)
    y = 64                    # softmax group size
    nx = xy // y              # x values (64)
    jt = xy // 2 // P         # u16-pair transpose tiles over xy (16)

    nc = bacc.Bacc("TRN2", target_bir_lowering=False, debug=False,
                   num_devices=n_cores)

    rgb = nc.dram_tensor("rgb", [c, hwc], F32, kind="ExternalInput")
    chm = nc.dram_tensor("chm", [c, xy], F32, kind="ExternalInput")
    wqT = nc.dram_tensor("wqT", [c, 2 * cqk], F32, kind="ExternalInput")
    wkT = nc.dram_tensor("wkT", [c, 2 * cqk], F32, kind="ExternalInput")
    wvT = nc.dram_tensor("wvT", [c, c], F32, kind="ExternalInput")
    bq = nc.dram_tensor("bq", [2 * cqk, 1], F32, kind="ExternalInput")
    bk = nc.dram_tensor("bk", [2 * cqk, 1], F32, kind="ExternalInput")
    ident = nc.dram_tensor("ident", [P, P], F32, kind="ExternalInput")
    out = nc.dram_tensor("out", [c, hwc], F32, kind="ExternalOutput")

    rgb_t = rgb.ap().rearrange("(k p) n -> p k n", p=P)
    chm_t = chm.ap().rearrange("(k p) n -> p k n", p=P)
    wq_t = wqT.ap().rearrange("(k p) m -> p k m", p=P)
    wk_t = wkT.ap().rearrange("(k p) m -> p k m", p=P)
    wv_t = wvT.ap().rearrange("(k p) m -> p k m", p=P)
    out_t = out.ap().rearrange("(k p) n -> p k n", p=P)

    with tile.TileContext(nc) as tc, \
         nc.allow_low_precision(reason="attention weights in fp8"):
        with tc.tile_pool(name="persist", bufs=1) as pers:
            # --- persistent tiles ---
            wv_b = pers.tile([P, ck, c], BF16)
            bq_sb = pers.tile([2 * cqk, 1], F32)
            bk_sb = pers.tile([2 * cqk, 1], F32)
            qt_sb = pers.tile([2 * cqk, hwc], BF16)
            kf_sb = pers.tile([2 * cqk, xy], BF16)    # y-major columns
            wq_b = pers.tile([P, ck, 2 * cqk], BF16)
            wk_b = pers.tile([P, ck, 2 * cqk], BF16)
            id_b = pers.tile([P, P], BF16)
            rgb_b = pers.tile([P, ck, hwc], BF16)
            chmT8c = pers.tile([P, ck, jt, 2, P], FP8)  # chm^T, canonical

            # --- phase 1 ---
            with tc.tile_pool(name="qload", bufs=2) as qload, \
                 tc.tile_pool(name="kload", bufs=2) as kload, \
                 tc.tile_pool(name="wload", bufs=1) as wload, \
                 tc.tile_pool(name="c8pool", bufs=2) as c8pool, \
                 tc.tile_pool(name="ctpool", bufs=2) as ctpool:
                wq_sb = wload.tile([P, ck, 2 * cqk], F32, name="wq_sb")
                wk_sb = wload.tile([P, ck, 2 * cqk], F32, name="wk_sb")
                id_sb = wload.tile([P, P], F32, name="id_sb")
                nc.scalar.dma_start(wq_sb[:], wq_t)
                nc.scalar.dma_start(wk_sb[:], wk_t)
                nc.scalar.dma_start(bq_sb[:], bq.ap())
                nc.scalar.dma_start(bk_sb[:], bk.ap())
                rfh = [qload.tile([P, 2, hwc], F32, tag="rfh",
                                  name=f"rfh{h}") for h in range(2)]
                cfh = [kload.tile([P, 2, xy], F32, tag="cfh",
                                  name=f"cfh{h}") for h in range(2)]
                cb = {}
                cf = [cfh[k // 2][:, k % 2] for k in range(ck)]

                wv_f = wload.tile([P, ck, c], F32)
                nc.scalar.dma_start(wv_f[:], wv_t)
                nc.scalar.dma_start(id_sb[:], ident.ap())
                nc.scalar.dma_start(rfh[0][:], rgb_t[:, 0:2])
                nc.scalar.dma_start(rfh[1][:], rgb_t[:, 2:4])
                nc.scalar.dma_start(cfh[0][:], chm_t[:, 0:2])
                nc.scalar.dma_start(cfh[1][:], chm_t[:, 2:4])
                nc.gpsimd.tensor_copy(wv_b[:], wv_f[:])
                nc.vector.tensor_copy(wq_b[:], wq_sb[:])
                nc.vector.tensor_copy(wk_b[:], wk_sb[:])
                nc.vector.tensor_copy(id_b[:], id_sb[:])
                for h in range(2):
                    # bf16 copy of rgb (DVE idle in phase 1)
                    nc.vector.tensor_copy(rgb_b[:, 2 * h:2 * h + 2],
                                          rfh[h][:])

                for k in range(ck):
                    # bf16 copies of chm chunk halves: K GEMM rhs + the
                    # chm^T transpose source (SP ring, as soon as ready)
                    cfv = cf[k][:].rearrange("p (xx yy) -> p yy xx", yy=y)
                    for h in range(2):
                        cb[k, h] = kload.tile([P, 32, nx], BF16, tag="cb",
                                              name=f"cb{k}_{h}")
                        nc.vector.tensor_copy(
                            cb[k, h][:], cfv[:, 32 * h:32 * (h + 1)])
                    # fp8 permuted copy (Pool), u16-pair transpose, then
                    # de-interleave to the canonical [K, s, M] weights
                    # layout on ACT (walrus rejects strided DR weights)
                    chm8k = c8pool.tile([P, xy], FP8, tag="chm8",
                                        name=f"chm8_{k}")
                    c8v = chm8k[:].rearrange("p (yy xx) -> p xx yy", xx=nx)
                    nc.gpsimd.tensor_copy(
                        c8v, cf[k][:].rearrange("p (xx yy) -> p xx yy",
                                                yy=y))
                    ct8 = ctpool.tile([P, jt, P], U16, tag="ct8",
                                      name=f"ct8_{k}")
                    nc.sync.dma_start(ct8[:], chm8k[:].bitcast(U16),
                                      transpose=True)
                    nc.gpsimd.tensor_copy(
                        chmT8c[:, k],
                        ct8[:].bitcast(FP8)
                        .rearrange("p m (cc ss) -> p m ss cc", ss=2))

                with tc.tile_pool(name="psQ", bufs=2, space="PSUM") as psQ:
                    # Q j-outer: one bank per 512-col block, 2 banks rotate
                    for j in range(nb):
                        q_ps = psQ.tile([2 * cqk, 512], F32, tag="qps",
                                        name="q_ps")
                        for k in range(ck):
                            nc.tensor.matmul(
                                q_ps[:], wq_b[:, k],
                                rgb_b[:, k, 512 * j:512 * (j + 1)],
                                start=(k == 0), stop=(k == ck - 1))
                        nc.scalar.activation(
                            qt_sb[:, 512 * j:512 * (j + 1)], q_ps[:],
                            IDENT, bias=bq_sb[:])

                with tc.tile_pool(name="psK", bufs=1, space="PSUM") as psK:
                    # K full-width k-outer in bf16 (8 banks), trailing the
                    # chm chunk loads + bf16 converts.
                    k_ps = [psK.tile([2 * cqk, 2048], F32, name=f"kps{i}")
                            for i in range(2)]
                    for k in range(ck):
                        for j in range(8):
                            nc.tensor.matmul(
                                k_ps[j // 4][:, 512 * (j % 4):
                                             512 * (j % 4 + 1)],
                                wk_b[:, k],
                                cb[k, j // 4][:]
                                .rearrange("p a b -> p (a b)")
                                [:, 512 * (j % 4):512 * (j % 4 + 1)],
                                start=(k == 0), stop=(k == ck - 1))
                    for i in range(2):
                        nc.scalar.activation(
                            kf_sb[:, 2048 * i:2048 * (i + 1)], k_ps[i][:],
                            IDENT, bias=bk_sb[:])

            # --- phase 2: softmax(b) / attend(b-2) pipeline ---
            with tc.tile_pool(name="pmain", bufs=3) as pmain, \
                 tc.tile_pool(name="pbpool", bufs=2) as pbpool, \
                 tc.tile_pool(name="p8pool", bufs=2) as p8pool, \
                 tc.tile_pool(name="ptpool", bufs=2) as ptpool, \
                 tc.tile_pool(name="zpool", bufs=1) as zpool, \
                 tc.tile_pool(name="opool", bufs=1) as opool, \
                 tc.tile_pool(name="m1pool", bufs=2) as m1pool, \
                 tc.tile_pool(name="psS", bufs=3, space="PSUM") as psS, \
                 tc.tile_pool(name="psA", bufs=1, space="PSUM") as psA:

                ptb = {}
                m1t = {}

                def m1_quarter(blk, ch, quarter):
                    """4 DoubleRow matmuls: one quarter of M1[ch, blk]."""
                    if blk not in m1t:
                        m1t[blk] = m1pool.tile([P, ck, 512], BF16, tag="m1",
                                               name=f"m1_{blk}")
                    if ch % 2 == 0 and quarter == 0:
                        m1t[(blk, "ps", ch // 2)] = psA.tile(
                            [P, 1024], F32, tag="aps", name="m1ps")
                    m_ps = m1t[(blk, "ps", ch // 2)]
                    half = m_ps[:, 512 * (ch % 2):512 * (ch % 2 + 1)]
                    for m in range(4 * quarter, 4 * quarter + 4):
                        rhs = (ptb[blk][:, m]
                               .rearrange("p hh nn -> p (hh nn)")
                               .bitcast(FP8)
                               .rearrange("p (nn ss) -> p ss nn", ss=2))
                        nc.tensor.matmul(
                            half, chmT8c[:, ch, m], rhs, perf_mode=DR,
                            start=(m == 0), stop=(m == jt - 1))
                    if ch % 2 == 1 and quarter == 3:
                        m_ps = m1t.pop((blk, "ps", ch // 2))
                        nc.scalar.activation(
                            m1t[blk][:, ch - 1:ch + 1], m_ps[:]
                            .rearrange("p (a b) -> p a b", a=2), IDENT)

                def softmax_htile(blk, ht, interleave, pool_convert):
                    """S matmuls + exp + normalize + fp8 transpose for one
                    128-row hw tile; S-chunk matmuls alternate with
                    `interleave` chunks (callables emitting PE work)."""
                    htile = blk * 4 + ht
                    p_sb = pmain.tile([P, xy], BF16, tag="p")
                    for s in range(xy // 1024):
                        s_ps = psS.tile([P, 1024], F32, tag="sps")
                        nc.tensor.matmul(
                            s_ps[:, 0:512],
                            qt_sb[0:cqk, P * htile:P * (htile + 1)],
                            kf_sb[0:cqk, 1024 * s:1024 * s + 512],
                            start=True, stop=True, tile_position=(0, 0))
                        nc.tensor.matmul(
                            s_ps[:, 512:1024],
                            qt_sb[cqk:2 * cqk, P * htile:P * (htile + 1)],
                            kf_sb[cqk:2 * cqk, 1024 * s + 512:1024 * (s + 1)],
                            start=True, stop=True, tile_position=(cqk, 0))
                        nc.scalar.activation(
                            p_sb[:, 1024 * s:1024 * (s + 1)], s_ps[:], EXP)
                        if s < len(interleave):
                            interleave[s]()
                    # Z[hw, x] = sum_y P_u[hw, y, x]: pairwise tree over the
                    # y (outer) dim keeps stride-1 innermost (DVE 2x).
                    v3 = p_sb[:].rearrange("p (yy xx) -> p yy xx", xx=nx)
                    tcur = v3
                    w = y
                    while w > 1:
                        w //= 2
                        tnext = zpool.tile([P, w, nx], BF16, tag=f"z{w}")
                        nc.vector.tensor_tensor(
                            tnext[:], tcur[:, 0:w], tcur[:, w:2 * w], ADD)
                        tcur = tnext
                    rz = zpool.tile([P, 1, nx], BF16, tag="rz")
                    nc.vector.reciprocal(rz[:], tcur[:])
                    p8 = p8pool.tile([P, y, nx], FP8, tag="p8")
                    if pool_convert:
                        # bf16 normalize at DVE 2x; Pool quantizes to fp8
                        pb = pbpool.tile([P, y, nx], BF16, tag="pb")
                        nc.vector.tensor_tensor(
                            pb[:], v3, rz[:].to_broadcast([P, y, nx]), MULT)
                        nc.gpsimd.tensor_copy(p8[:], pb[:])
                    else:
                        # normalize + quantize to fp8 in one DVE pass
                        nc.vector.tensor_tensor(
                            p8[:], v3, rz[:].to_broadcast([P, y, nx]), MULT)
                    if blk not in ptb:
                        # [p, m, ht, hw]: hw contiguous across ht per m so
                        # the M1 rhs merges to one 512-wide moving AP.
                        ptb[blk] = ptpool.tile([P, jt, 4, P], U16,
                                               tag="ptb", name=f"ptb{blk}")
                    nc.sync.dma_start(ptb[blk][:, :, ht], p8[:].bitcast(U16),
                                      transpose=True)

                def attend_fini(blk):
                    """att2 + rgb add (via identity matmul) + store."""
                    m1_sb = m1t.pop(blk)
                    o_sb = opool.tile([P, ck, 512], F32, tag="o")
                    for cp in range(2):
                        a_ps = psA.tile([P, 1024], F32, tag="aps",
                                        name="a_ps")
                        for h in range(2):
                            ct = 2 * cp + h
                            half = a_ps[:, 512 * h:512 * (h + 1)]
                            for chh in range(ck):
                                nc.tensor.matmul(
                                    half, wv_b[:, chh, P * ct:P * (ct + 1)],
                                    m1_sb[:, chh],
                                    start=(chh == 0), stop=False)
                            nc.tensor.matmul(
                                half, id_b[:],
                                rgb_b[:, ct, 512 * blk:512 * (blk + 1)],
                                start=False, stop=True)
                        nc.scalar.activation(
                            o_sb[:, 2 * cp:2 * cp + 2],
                            a_ps[:].rearrange("p (a b) -> p a b", a=2), IDENT)
                    nc.sync.dma_start(
                        out_t[:, :, 512 * blk:512 * (blk + 1)], o_sb[:])
                    del ptb[blk]

                def slot(ab, ch):
                    return [(lambda c=ch, q=q: m1_quarter(ab, c, q))
                            for q in range(4)]

                # depth-1.5 pipeline: softmax(b) hosts the second half of
                # attend(b-2), its fini, then the first half of attend(b-1)
                for blk in range(nb):
                    for ht in range(4):
                        ab, ch = blk - 2 + ht // 2, (ht % 2) + 2 * (ht // 2 == 0)
                        ivl = slot(ab, ch) if 0 <= ab < nb else []
                        softmax_htile(blk, ht, ivl, pool_convert=ht < 2)
                        if blk >= 2 and ht == 1:
                            attend_fini(blk - 2)
                # tail: finish attend(nb-2) and attend(nb-1)
                for ch in range(2, ck):
                    for q in range(4):
                        m1_quarter(nb - 2, ch, q)
                attend_fini(nb - 2)
                for ch in range(ck):
                    for q in range(4):
                        m1_quarter(nb - 1, ch, q)
                attend_fini(nb - 1)

    nc.compile()
    return nc


_NC_CACHE = {}


def _get_nc():
    if "nc" not in _NC_CACHE:
        _NC_CACHE["nc"] = build_program()
    return _NC_CACHE["nc"]


def make_in_maps(rgb_features, chm_features, Wq, bq, Wk, bk, Wv, bv, gamma):
    rgb_features = np.asarray(rgb_features, dtype=np.float32)
    chm_features = np.asarray(chm_features, dtype=np.float32)
    Wq = np.asarray(Wq, dtype=np.float32)
    Wk = np.asarray(Wk, dtype=np.float32)
    Wv = np.asarray(Wv, dtype=np.float32)
    bq = np.asarray(bq, dtype=np.float32)
    bk = np.asarray(bk, dtype=np.float32)
    bv = np.asarray(bv, dtype=np.float32)
    g = float(np.asarray(gamma).reshape(-1)[0])

    wqT = np.ascontiguousarray(np.concatenate([Wq.T, Wq.T], axis=1))
    wkT = np.ascontiguousarray(np.concatenate([Wk.T, Wk.T], axis=1))
    wvT = np.ascontiguousarray((g * Wv).T)
    # softmax rows sum to 1 per (hw, x); summing over the 64 x's makes the
    # bias term contribute exactly 64*gamma*bv[c] to every output pixel.
    rgb_adj = rgb_features + (64.0 * g * bv)[None, :, None, None]
    bq2 = np.ascontiguousarray(np.concatenate([bq, bq]).reshape(2 * CQK, 1))
    bk2 = np.ascontiguousarray(np.concatenate([bk, bk]).reshape(2 * CQK, 1))
    ident = np.eye(P, dtype=np.float32)

    in_maps = []
    for core in range(N_CORES):
        b, half = divmod(core, 2)
        rgb_c = np.ascontiguousarray(
            rgb_adj[b].reshape(C, HW)[:, half * HWC:(half + 1) * HWC])
        chm_c = np.ascontiguousarray(chm_features[b].reshape(C, HW))
        in_maps.append({
            "rgb": rgb_c, "chm": chm_c,
            "wqT": wqT, "wkT": wkT, "wvT": wvT,
            "bq": bq2, "bk": bk2, "ident": ident,
        })
    return in_maps


def assemble(results):
    fused = np.empty((B, C, H, W), dtype=np.float32)
    fused2 = fused.reshape(B, C, HW)
    for core in range(N_CORES):
        b, half = divmod(core, 2)
        fused2[b, :, half * HWC:(half + 1) * HWC] = results[core]["out"]
    return fused


def kernel(rgb_features, chm_features, Wq, bq, Wk, bk, Wv, bv, gamma):
    nc = _get_nc()
    in_maps = make_in_maps(rgb_features, chm_features, Wq, bq, Wk, bk, Wv, bv,
                           gamma)
    res = run_bass_kernel_spmd(nc, in_maps, core_ids=list(range(N_CORES)))
    return assemble(res.results)
